# revision 1
# baseline (speedup 1.0000x reference)
"""Multi-head causal attention (B=4, S=2048, D=1024, H=16) on 8 NeuronCores.

Sharding: core c handles batch b = c//2 and head-group g = c%2 (8 heads).
Each core computes QKV projections for its group, causal attention for its
8 heads, and a partial output projection (row-split Wo).  Host sums the two
partials per batch and adds bo.

On-chip design (per core), all matmuls fp16 with fp32 PSUM accumulation:
  XT [D, S] = X[b].T in 8 chunks [128, S].
  QT/KT pair-tiles [128, S]: partitions 0-63 = head 2c, 64-127 = head 2c+1,
  computed as W.T-chunk (stationary) x XT (moving), bias added during PSUM
  evacuation on ScalarE (per-partition bias AP).
  V per s-chunk [128, 8, 66]: 64 V cols (+bv via broadcast tile) then
  [1,0] (even heads) / [0,1] (odd heads) columns so the PV matmul emits
  rowsum(exp(scores)) at psum row 64 / 65.
  scoresT tiles [sk=128, sq<=512] via two row-tiled K=64 matmuls (head pair
  shares the PE array, separate PSUM banks).  Causal masking: windowed
  matmuls skip fully-masked regions; diagonal 128x128 blocks get -30000
  added via an identity x mask matmul accumulate (keeps exp's deps on PE
  only).  exp on ScalarE (no max subtraction; |scores| <= ~3).
  Normalization: rowsums DMA-scattered across 128 partitions, DVE
  reciprocal, DMA-gathered back to partitions 64-65, broadcast down
  partitions with a K=2 indicator matmul (tile_position row 64), folded
  into OT via one tensor_mul per 512-slice.

Walrus wait-slot legality (1 sem wait per ACT/DVE/DMA instruction): touch
ops pre-observe constant DMAs, reused DVE-written tiles are pre-memset so
the memset absorbs the WAR wait, and the normalization tiles use
one-buffer-per-pair pools so slots are never reused.
"""

import sys

for _p in ("/opt/trn_rl_repo",):
    if _p not in sys.path:
        sys.path.insert(0, _p)

from contextlib import ExitStack

import numpy as np

import concourse.bass as bass
import concourse.mybir as mybir
import concourse.tile as tile
from concourse.bass_utils import run_bass_kernel_spmd

import bass_rust

F16 = mybir.dt.float16
F32 = mybir.dt.float32
AF = mybir.ActivationFunctionType

B, S, D, H = 4, 2048, 1024, 16
HD = D // H  # 64
GH = 8  # heads per group
GW = GH * HD  # 512 columns per group


_SPLITTABLE = {
    "InstMatmult", "InstLdweights", "InstActivation", "InstTensorCopy",
    "InstTensorTensor", "InstTensorScalarPtr", "InstTensorReduce",
    "InstMemset", "InstDMACopy", "InstReciprocal", "InstIota",
    "InstTensorTensorReduce", "InstBNStats", "InstBNStatsAggregate",
    "InstStreamShuffle", "InstNoOp", "InstPool", "InstMax", "InstDrain",
}


def _legalize_waits(nc, max_waits=1):
    """Walrus codegen accepts at most one sync-wait command per engine
    instruction; Tile's wait assigner can emit more.  Split extras onto
    same-engine NoOps inserted immediately before (semantics preserved:
    the engine blocks at the same program point)."""
    ctr = 0
    for fn in nc.m.functions:
        for blk in fn.blocks:
            out = []
            for ins in blk.instructions:
                si = ins.sync_info
                if (
                    si is not None
                    and len(si.on_wait) > max_waits
                    and type(ins).__name__ in _SPLITTABLE
                ):
                    waits = list(si.on_wait)
                    extra, keep = waits[:-max_waits], waits[-max_waits:]
                    for w in extra:
                        nop = mybir.InstNoOp(name=f"waitnop-{ctr}", ins=[], outs=[])
                        ctr += 1
                        nop.engine = ins.engine
                        nop.sync_info = bass_rust.SyncInfo(on_wait=[w], on_update=[])
                        out.append(nop)
                    ins.sync_info = bass_rust.SyncInfo(
                        on_wait=keep, on_update=list(si.on_update)
                    )
                out.append(ins)
            blk.instructions[:] = out
    return ctr


def build_nc(s=S, legalize=True, reps=1):
    ns = s // 512  # 512-wide sq slices per head
    nt = s // 128  # 128-wide s chunks
    nd = D // 128  # contraction chunks for projections
    nb = ns * 512 // 32  # 32-elem blocks per rowsum row (scatter layout)

    nc = bass.Bass("TRN2", target_bir_lowering=False, debug=False)
    xt_d = nc.dram_tensor("xt", [D, s], F16, kind="ExternalInput").ap()
    wq_d = nc.dram_tensor("wq", [D, GW], F16, kind="ExternalInput").ap()
    wk_d = nc.dram_tensor("wk", [D, GW], F16, kind="ExternalInput").ap()
    wv_d = nc.dram_tensor("wv", [D, GW], F16, kind="ExternalInput").ap()
    wo_d = nc.dram_tensor("wo", [GW, D], F16, kind="ExternalInput").ap()
    bqk_d = nc.dram_tensor("bqk", [128, 8], F32, kind="ExternalInput").ap()
    bvb_d = nc.dram_tensor("bvb", [128, GW], F16, kind="ExternalInput").ap()
    mask_d = nc.dram_tensor("mask", [128, 128], F16, kind="ExternalInput").ap()
    out_d = nc.dram_tensor("out", [s, D], F32, kind="ExternalOutput").ap()

    with tile.TileContext(nc) as tc, ExitStack() as ctx:
        pool = lambda name, bufs, **kw: ctx.enter_context(
            tc.tile_pool(name=name, bufs=bufs, **kw)
        )
        const_p = pool("const", 1)
        xt_p = pool("xtp", nd)
        w_p = pool("wp", 1)
        qt_p = pool("qtp", 4)
        kt_p = pool("ktp", 4)
        v_p = pool("vp", nt)
        et_p = pool("etp", 6)
        ot_p = pool("otp", 4)
        rs_p = pool("rsp", 4)
        tmp_p = pool("tmpp", 4)
        ob_p = pool("obp", 4)
        ps_proj = pool("psproj", 2, space="PSUM")
        ps_qk = pool("psqk", 2, space="PSUM")
        ps_pv = pool("pspv", 2, space="PSUM")

        for _rep in range(reps):
            # --- inputs, in order of first use: wv + xt feed the V
            # projection, then bvb, wq/wk, bqk, mask; wo only at the end ---
            wq_sb = w_p.tile([128, nd, GW], F16)
            wk_sb = w_p.tile([128, nd, GW], F16)
            wv_sb = w_p.tile([128, nd, GW], F16)
            wo_sb = w_p.tile([128, 4, D], F16)
            nc.sync.dma_start(out=wv_sb[:], in_=wv_d.rearrange("(d p) n -> p d n", p=128))
            xt_sb = []
            for d in range(nd):
                t = xt_p.tile([128, s], F16, tag="xt", name=f"xtc{d}")
                nc.sync.dma_start(out=t[:], in_=xt_d[d * 128 : (d + 1) * 128, :])
                xt_sb.append(t)
            bvb_sb = const_p.tile([128, GW], F16)
            nc.sync.dma_start(out=bvb_sb[:], in_=bvb_d[:])
            nc.sync.dma_start(out=wq_sb[:], in_=wq_d.rearrange("(d p) n -> p d n", p=128))
            nc.sync.dma_start(out=wk_sb[:], in_=wk_d.rearrange("(d p) n -> p d n", p=128))
            bqk_sb = const_p.tile([128, 8], F32)
            nc.sync.dma_start(out=bqk_sb[:], in_=bqk_d[:])
            mask_sb = const_p.tile([128, 128], F16)
            nc.sync.dma_start(out=mask_sb[:], in_=mask_d[:])
            nc.sync.dma_start(out=wo_sb[:], in_=wo_d.rearrange("(c p) n -> p c n", p=128))

            # touch ops: early ACT-table load + const observations
            scr_a = const_p.tile([128, 1], F32)
            nc.scalar.copy(scr_a[:], bqk_sb[:, 0:1])
            scr_v = const_p.tile([128, 1], F16)
            nc.vector.tensor_copy(scr_v[:], bvb_sb[:, 0:1])
            scr_m = const_p.tile([128, 1], F16)
            nc.vector.tensor_copy(scr_m[:], mask_sb[:, 0:1])

            # --- Q/K projections: QT/KT pair-tiles [128, s] ---
            qt_sb = [qt_p.tile([128, s], F16, tag="qt", name=f"qt{c}") for c in range(4)]
            kt_sb = [kt_p.tile([128, s], F16, tag="kt", name=f"kt{c}") for c in range(4)]
            ot_sb = [ot_p.tile([128, s], F16, tag="ot", name=f"ot{c}") for c in range(4)]
            # --- V projection: per s-chunk [128, 8, 66] with rowsum cols ---
            v_sb = []
            for st in range(nt):
                ps = ps_proj.tile([128, 512], F32, tag="ps", name="ps")
                for d in range(nd):
                    nc.tensor.matmul(
                        ps[:],
                        xt_sb[d][:, st * 128 : (st + 1) * 128],
                        wv_sb[:, d, :],
                        start=(d == 0),
                        stop=(d == nd - 1),
                    )
                vt = v_p.tile([128, GH, 66], F16, tag="v", name=f"v{st}")
                nc.vector.memset(vt[:, 0::2, 64:65], 1.0)
                nc.vector.memset(vt[:, 1::2, 64:65], 0.0)
                nc.vector.memset(vt[:, 1::2, 65:66], 1.0)
                nc.vector.tensor_add(
                    vt[:, :, 0:64],
                    ps[:].rearrange("p (h e) -> p h e", h=GH),
                    bvb_sb[:].rearrange("p (h e) -> p h e", h=GH),
                )
                v_sb.append(vt)

            # --- per pair: Q/K projection then attention ---
            for c in range(4):
                for dst, wsb, bcol in ((qt_sb[c], wq_sb, c), (kt_sb[c], wk_sb, 4 + c)):
                    for sl in range(ns):
                        ps = ps_proj.tile([128, 512], F32, tag="ps", name="ps")
                        for d in range(nd):
                            nc.tensor.matmul(
                                ps[:],
                                wsb[:, d, c * 128 : (c + 1) * 128],
                                xt_sb[d][:, sl * 512 : (sl + 1) * 512],
                                start=(d == 0),
                                stop=(d == nd - 1),
                            )
                        nc.vector.tensor_scalar_add(
                            dst[:, sl * 512 : (sl + 1) * 512],
                            ps[:],
                            bqk_sb[:, bcol : bcol + 1],
                        )

                for j in range(ns):
                    stage = rs_p.tile([66, 512], F16, tag="rs", name="stage")
                    pv0 = ps_pv.tile([128, 512], F32, tag="pv", name="pv0")
                    pv1 = ps_pv.tile([128, 512], F32, tag="pv", name="pv1")
                    last = 4 * j + 3
                    for t in range(last + 1):
                        diag = t >= 4 * j
                        w0 = 128 * (t - 4 * j) if diag else 0
                        qk = ps_qk.tile([128, 2, 512], F32, tag="qk", name="qk")
                        for hh in range(2):
                            nc.tensor.matmul(
                                qk[:, hh, w0:512],
                                kt_sb[c][hh * 64 : hh * 64 + 64, t * 128 : (t + 1) * 128],
                                qt_sb[c][
                                    hh * 64 : hh * 64 + 64, j * 512 + w0 : (j + 1) * 512
                                ],
                                start=True,
                                stop=True,
                            )
                        et = et_p.tile([128, 2, 512], F16, tag="et", name="et")
                        nc.scalar.activation(
                            et[:, :, w0:512], qk[:, :, w0:512], AF.Exp, scale=0.125
                        )
                        if diag:
                            # zero the masked (upper) triangle of the diagonal
                            # 128x128 block with a 0/1 mask multiply
                            for hh in range(2):
                                nc.vector.tensor_mul(
                                    et[:, hh, w0 : w0 + 128],
                                    et[:, hh, w0 : w0 + 128],
                                    mask_sb[:],
                                )
                        nc.tensor.matmul(
                            pv0[0:65, w0:512],
                            v_sb[t][:, 2 * c, 0:65],
                            et[:, 0, w0:512],
                            start=(t == 0),
                            stop=(t == last),
                        )
                        nc.tensor.matmul(
                            pv1[0:66, w0:512],
                            v_sb[t][:, 2 * c + 1, 0:66],
                            et[:, 1, w0:512],
                            start=(t == 0),
                            stop=(t == last),
                        )
                    # evacuate attention outputs (unnormalized) + rowsums
                    nc.vector.tensor_copy(
                        ot_sb[c][0:64, j * 512 : (j + 1) * 512], pv0[0:64, :]
                    )
                    todd = tmp_p.tile([64, 512], F16, tag="todd", name="todd")
                    nc.vector.tensor_copy(todd[:], pv1[0:64, :])
                    nc.sync.dma_start(
                        out=ot_sb[c][64:128, j * 512 : (j + 1) * 512], in_=todd[:]
                    )
                    nc.vector.tensor_copy(stage[64:66, :], pv1[64:66, :])
                    nc.vector.tensor_copy(stage[64:65, :], pv0[64:65, :])
                    # rowsum reciprocals for this slice: scatter [2, 512]
                    # across 32 partitions, DVE-reciprocal, gather back,
                    # then replicate down all 128 partitions via DMA
                    rs128 = rs_p.tile([32, 32], F16, tag="rs128", name="rs128")
                    nc.sync.dma_start(
                        out=rs128[:],
                        in_=stage[64:66, :].rearrange("p (b e) -> p b e", e=32),
                    )
                    r128 = rs_p.tile([32, 32], F16, tag="r128", name="r128")
                    with nc.allow_low_precision(reason="softmax divisor"):
                        nc.vector.reciprocal(r128[:], rs128[:])
                    rsr = rs_p.tile([66, 512], F16, tag="rsr", name="rsr")
                    nc.sync.dma_start(
                        out=rsr[64:66, :].rearrange("p (b e) -> p b e", e=32),
                        in_=r128[:],
                    )
                    bcast = rs_p.tile([128, 512], F16, tag="bc", name="bcast")
                    rr = rsr[64:66, :]
                    nc.sync.dma_start(
                        out=bcast[:],
                        in_=bass.AP(
                            tensor=rr.tensor,
                            offset=rr.offset,
                            ap=[rr.ap[0], [0, 64], [1, 512]],
                        ),
                    )
                    nc.vector.tensor_mul(
                        ot_sb[c][:, j * 512 : (j + 1) * 512],
                        ot_sb[c][:, j * 512 : (j + 1) * 512],
                        bcast[:],
                    )
                    if c == 3:
                        # all four chunks of this j-slice are normalized:
                        # emit the output projection for its s-tiles.  For
                        # the final slice (strictly post-attention) spread
                        # the accumulators over the idle qk/pv PSUM pools so
                        # six groups can pre-accumulate chunks 0-2 while
                        # chunk 3's normalization chain completes.
                        po_pools = (
                            [(ps_proj, "ps"), (ps_qk, "qk"), (ps_pv, "pv")]
                            if j == ns - 1
                            else [(ps_proj, "ps")]
                        )
                        for sti, st in enumerate(range(4 * j, 4 * j + 4)):
                            for dsl in range(2):
                                pp, ptag = po_pools[
                                    (sti * 2 + dsl) % len(po_pools)
                                ]
                                po = pp.tile(
                                    [128, 512], F32, tag=ptag, name="po"
                                )
                                for cc in range(4):
                                    nc.tensor.matmul(
                                        po[:],
                                        ot_sb[cc][:, st * 128 : (st + 1) * 128],
                                        wo_sb[:, cc, dsl * 512 : (dsl + 1) * 512],
                                        start=(cc == 0),
                                        stop=(cc == 3),
                                    )
                                ob = ob_p.tile([128, 512], F32, tag="ob", name="ob")
                                nc.vector.tensor_copy(ob[:], po[:])
                                nc.sync.dma_start(
                                    out=out_d[
                                        st * 128 : (st + 1) * 128,
                                        dsl * 512 : (dsl + 1) * 512,
                                    ],
                                    in_=ob[:],
                                )

    if legalize:
        _legalize_waits(nc)
    return nc


_NC_CACHE = {}


def _get_nc(s=S):
    if s not in _NC_CACHE:
        _NC_CACHE[s] = build_nc(s)
    return _NC_CACHE[s]


def make_inputs(X, Wq, bq, Wk, bk, Wv, bv, Wo, bo, s=S):
    """Per-core input maps. Core c: batch c//2, head group c%2."""
    iv, jv = np.arange(128)[:, None], np.arange(128)[None, :]
    mask = (jv >= iv).astype(np.float16)
    in_maps = []
    for c in range(8):
        b, g = divmod(c, 2)
        lo, hi = g * GW, (g + 1) * GW
        bqk = np.concatenate(
            [
                np.ascontiguousarray(bq[lo:hi].reshape(4, 128).T),
                np.ascontiguousarray(bk[lo:hi].reshape(4, 128).T),
            ],
            axis=1,
        ).astype(np.float32)
        in_maps.append(
            {
                "xt": np.ascontiguousarray(X[b, :s].T).astype(np.float16),
                "wq": np.ascontiguousarray(Wq[lo:hi].T).astype(np.float16),
                "wk": np.ascontiguousarray(Wk[lo:hi].T).astype(np.float16),
                "wv": np.ascontiguousarray(Wv[lo:hi].T).astype(np.float16),
                "wo": np.ascontiguousarray(Wo[:, lo:hi].T).astype(np.float16),
                "bqk": bqk,
                "bvb": np.tile(bv[lo:hi].astype(np.float16), (128, 1)),
                "mask": mask,
            }
        )
    return in_maps


def kernel(X, Wq, bq, Wk, bk, Wv, bv, Wo, bo, **run_kwargs):
    args = [np.asarray(a, np.float32) for a in (X, Wq, bq, Wk, bk, Wv, bv, Wo, bo)]
    X, Wq, bq, Wk, bk, Wv, bv, Wo, bo = args
    nc = _get_nc(S)
    in_maps = make_inputs(X, Wq, bq, Wk, bk, Wv, bv, Wo, bo, S)
    res = run_bass_kernel_spmd(nc, in_maps, core_ids=list(range(8)), **run_kwargs)
    outs = [r["out"] for r in res.results]
    full = np.empty((B, S, D), np.float32)
    for b in range(B):
        full[b] = outs[2 * b] + outs[2 * b + 1] + bo
    kernel.last_results = res
    return full



# revision 32
# speedup vs baseline: 1.1939x; 1.1939x over previous
"""Multi-head causal attention (B=4, S=2048, D=1024, H=16) on 8 NeuronCores.

Sharding: core c handles batch b = c//2 and head-group g = c%2 (8 heads).
Each core computes QKV projections for its group, causal attention for its
8 heads, and a partial output projection (row-split Wo).  Host sums the two
fp16 partials per batch and adds bo.

On-chip design (per core), all matmuls fp16 with fp32 PSUM accumulation.
Engines execute in-order, so the emission IS the schedule: the attention
t-loop is ACT-bound (exp ~1.04us/t vs ~0.65us/t of PE work), so projection
work is decomposed into single-matmul "filler" units woven ~2 per t-step
into the attention stream, qk(t+1) is emitted before pv(t) so the wait on
exp(t) lands as late as possible, and each slice's normalization is
deferred into the next slice's stream.

  xt [D, S] = X[b].T in four [128, 8, 512] column blocks (block 0 and wv
  stream as interleaved per-d pieces so compute starts after ~1 MB of DMA).
  QT/KT pair-tiles [128, S]: partitions 0-63 = head 2c, 64-127 = head 2c+1.
  V per s-chunk [128, 8, 65]: 64 V cols (+bv) plus a ones column so every
  PV matmul emits rowsum(exp(scores)) in output column 64.
  scoresT tiles [sk=128, sq<=512] via two K=64 matmuls; windowed matmuls
  skip fully-masked regions; diagonal 128x128 blocks masked by a 0/1
  tensor_mul on the exp output.
  PV is TRANSPOSED vs the classic layout: stationary = dense et chunk
  [sk=128, sq=128], moving = V [sk=128, 65], accumulating psum pv[sq, 65]
  per (head, sq-chunk) over k-tiles: 65 moving columns per tile instead of
  512 (LdWeights is pipelined).  Rowsums land at pv col 64 on the same
  partition as their query, so normalization is one reciprocal [128, 2, 4]
  plus one broadcast tensor_mul per (pair, slice) into a normalized fp16
  stage [128, 512] = [sq, headA|headB]; PE transposes flip each 128-chunk
  into [hd-pair, sq] PSUM and one DVE copy lands ot_sb[c][:, slice] - the
  stationary layout the output projection needs.
  Output projection tiles run as fillers once their slice's four pairs are
  done; the final slice pre-accumulates cc=0..2 and fine-grains pair 3's
  evacuation per 128-chunk to shorten the tail.  Partials stored fp16.

Walrus wait-slot legality (1 sem wait per ACT/DVE/DMA instruction): touch
ops pre-observe constant DMAs and a legalization pass splits excess waits
onto same-engine NoOps.
"""

import sys

for _p in ("/opt/trn_rl_repo",):
    if _p not in sys.path:
        sys.path.insert(0, _p)

from collections import deque
from contextlib import ExitStack

import numpy as np

import concourse.bass as bass
import concourse.mybir as mybir
import concourse.tile as tile
from concourse.bass_utils import run_bass_kernel_spmd

import bass_rust

F16 = mybir.dt.float16
F32 = mybir.dt.float32
AF = mybir.ActivationFunctionType

B, S, D, H = 4, 2048, 1024, 16
HD = D // H  # 64
GH = 8  # heads per group
GW = GH * HD  # 512 columns per group


_SPLITTABLE = {
    "InstMatmult", "InstLdweights", "InstActivation", "InstTensorCopy",
    "InstTensorTensor", "InstTensorScalarPtr", "InstTensorReduce",
    "InstMemset", "InstDMACopy", "InstReciprocal", "InstIota",
    "InstTensorTensorReduce", "InstBNStats", "InstBNStatsAggregate",
    "InstStreamShuffle", "InstNoOp", "InstPool", "InstMax", "InstDrain",
}


def _legalize_waits(nc, max_waits=1):
    """Walrus codegen accepts at most one sync-wait command per engine
    instruction; Tile's wait assigner can emit more.  Split extras onto
    same-engine NoOps inserted immediately before (semantics preserved:
    the engine blocks at the same program point)."""
    ctr = 0
    for fn in nc.m.functions:
        for blk in fn.blocks:
            out = []
            for ins in blk.instructions:
                si = ins.sync_info
                if (
                    si is not None
                    and len(si.on_wait) > max_waits
                    and type(ins).__name__ in _SPLITTABLE
                ):
                    waits = list(si.on_wait)
                    extra, keep = waits[:-max_waits], waits[-max_waits:]
                    for w in extra:
                        nop = mybir.InstNoOp(name=f"waitnop-{ctr}", ins=[], outs=[])
                        ctr += 1
                        nop.engine = ins.engine
                        nop.sync_info = bass_rust.SyncInfo(on_wait=[w], on_update=[])
                        out.append(nop)
                    ins.sync_info = bass_rust.SyncInfo(
                        on_wait=keep, on_update=list(si.on_update)
                    )
                out.append(ins)
            blk.instructions[:] = out
    return ctr


def build_nc(s=S, legalize=True, pump_n=2):
    ns = s // 512  # 512-wide sq slices per head
    nt = s // 128  # 128-wide k tiles
    nd = D // 128  # contraction chunks for projections
    nb = s // 512  # xt column blocks

    nc = bass.Bass("TRN2", target_bir_lowering=False, debug=False)
    xt_d = nc.dram_tensor("xt", [D, s], F16, kind="ExternalInput").ap()
    wq_d = nc.dram_tensor("wq", [D, GW], F16, kind="ExternalInput").ap()
    wk_d = nc.dram_tensor("wk", [D, GW], F16, kind="ExternalInput").ap()
    wv_d = nc.dram_tensor("wv", [D, GW], F16, kind="ExternalInput").ap()
    wo_d = nc.dram_tensor("wo", [GW, D], F16, kind="ExternalInput").ap()
    bqk_d = nc.dram_tensor("bqk", [128, 8], F32, kind="ExternalInput").ap()
    bvb_d = nc.dram_tensor("bvb", [128, GW], F16, kind="ExternalInput").ap()
    mask_d = nc.dram_tensor("mask", [128, 128], F16, kind="ExternalInput").ap()
    idn_d = nc.dram_tensor("idn", [128, 128], F16, kind="ExternalInput").ap()
    out_d = nc.dram_tensor("out", [s, D], F16, kind="ExternalOutput").ap()

    with tile.TileContext(nc) as tc, ExitStack() as ctx:
        pool = lambda name, bufs, **kw: ctx.enter_context(
            tc.tile_pool(name=name, bufs=bufs, **kw)
        )
        const_p = pool("const", 1)
        xt_p = pool("xtp", nb)
        w_p = pool("wp", 1)
        qt_p = pool("qtp", 4)
        kt_p = pool("ktp", 4)
        v_p = pool("vp", nt)
        et_p = pool("etp", 6)
        ot_p = pool("otp", 4)
        st_p = pool("stp", 2)
        rc_p = pool("rcp", 2)
        ob_p = pool("obp", 4)
        ps_proj = pool("psproj", 2, space="PSUM")  # [128,512]f32    -> 2 banks
        ps_qk = pool("psqk", 2, space="PSUM")      # [128,2,512]f32  -> 4 banks
        ps_pv = pool("pspv", 1, space="PSUM")      # [128,2,512]f32  -> 2 banks

        # --- input DMAs, ordered by first use.  wv and xt block 0 stream as
        # interleaved per-d pieces so V-proj st0's d-th matmul can fire as
        # soon as its two pieces land. ---
        wv_sb = w_p.tile([128, nd, GW], F16)
        xt_sb = [
            xt_p.tile([128, nd, 512], F16, tag="xt", name=f"xtb{b}")
            for b in range(nb)
        ]
        bvb_sb = const_p.tile([128, GW], F16)
        wq_sb = w_p.tile([128, nd, GW], F16)
        wk_sb = w_p.tile([128, nd, GW], F16)
        wo_sb = w_p.tile([128, 4, D], F16)
        bqk_sb = const_p.tile([128, 8], F32)
        mask_sb = const_p.tile([128, 128], F16)
        idn_sb = const_p.tile([128, 128], F16)

        # graduated piece sizes: each dma_start costs ~0.5us of serialized
        # SP sequencer time, so only the leading pieces are fine-grained
        for a, b in ((0, 1), (1, 2), (2, 4), (4, 8)):
            nc.sync.dma_start(
                out=wv_sb[:, a:b, :],
                in_=wv_d[a * 128 : b * 128, :].rearrange("(d p) n -> p d n", p=128),
            )
            nc.sync.dma_start(
                out=xt_sb[0][:, a:b, :],
                in_=xt_d[a * 128 : b * 128, 0:512].rearrange(
                    "(d p) n -> p d n", p=128
                ),
            )
        nc.sync.dma_start(out=wq_sb[:], in_=wq_d.rearrange("(d p) n -> p d n", p=128))
        nc.sync.dma_start(out=wk_sb[:], in_=wk_d.rearrange("(d p) n -> p d n", p=128))
        for t, src2 in ((bqk_sb, bqk_d), (bvb_sb, bvb_d), (mask_sb, mask_d),
                        (idn_sb, idn_d)):
            nc.sync.dma_start(out=t[:], in_=src2[:])
        for bI in range(1, nb):
            nc.sync.dma_start(
                out=xt_sb[bI][:],
                in_=xt_d[:, bI * 512 : (bI + 1) * 512].rearrange(
                    "(d p) n -> p d n", p=128
                ),
            )
        nc.sync.dma_start(out=wo_sb[:], in_=wo_d.rearrange("(c p) n -> p c n", p=128))

        # touch ops: early Exp-table load + const observations
        scr_a = const_p.tile([128, 1], F32)
        nc.scalar.activation(scr_a[:], bqk_sb[:, 0:1], AF.Exp)
        scr_v = const_p.tile([128, 1], F16)
        nc.vector.tensor_copy(scr_v[:], bvb_sb[:, 0:1])
        scr_m = const_p.tile([128, 1], F16)
        nc.vector.tensor_copy(scr_m[:], mask_sb[:, 0:1])
        scr_i = const_p.tile([128, 1], F16)
        nc.vector.tensor_copy(scr_i[:], idn_sb[:, 0:1])

        qt_sb = [qt_p.tile([128, s], F16, tag="qt", name=f"qt{c}") for c in range(4)]
        kt_sb = [kt_p.tile([128, s], F16, tag="kt", name=f"kt{c}") for c in range(4)]
        ot_sb = [ot_p.tile([128, s], F16, tag="ot", name=f"ot{c}") for c in range(4)]
        v_sb = [None] * nt

        # ---------- filler generators: one PE matmul (~0.2us) per yield ----
        def gen_vproj(st):
            ps = ps_proj.tile([128, 512], F32, tag="ps", name="ps")
            blk, col = st // 4, (st % 4) * 128
            for d in range(nd):
                nc.tensor.matmul(
                    ps[:],
                    xt_sb[blk][:, d, col : col + 128],
                    wv_sb[:, d, :],
                    start=(d == 0),
                    stop=(d == nd - 1),
                )
                if d < nd - 1:
                    yield
            vt = v_p.tile([128, GH, 65], F16, tag="v", name=f"v{st}")
            nc.vector.memset(vt[:, :, 64:65], 1.0)
            nc.vector.tensor_add(
                vt[:, :, 0:64],
                ps[:].rearrange("p (h e) -> p h e", h=GH),
                bvb_sb[:].rearrange("p (h e) -> p h e", h=GH),
            )
            v_sb[st] = vt

        def gen_qkproj(c, sl):
            for dst, wsb, bcol in ((qt_sb[c], wq_sb, c), (kt_sb[c], wk_sb, 4 + c)):
                ps = ps_proj.tile([128, 512], F32, tag="ps", name="ps")
                for d in range(nd):
                    nc.tensor.matmul(
                        ps[:],
                        wsb[:, d, c * 128 : (c + 1) * 128],
                        xt_sb[sl][:, d, :],
                        start=(d == 0),
                        stop=(d == nd - 1),
                    )
                    if d < nd - 1:
                        yield
                nc.vector.tensor_scalar_add(
                    dst[:, sl * 512 : (sl + 1) * 512],
                    ps[:],
                    bqk_sb[:, bcol : bcol + 1],
                )
                yield

        def out_proj(st, dsl, pp, ptag, ccs, po=None):
            """Accumulate output projection for s-tile st, D-half dsl over
            pair-chunks ccs; evacuate + DMA when 3 in ccs."""
            if po is None:
                po = pp.tile([128, 512], F32, tag=ptag, name="po")
            for cc in ccs:
                nc.tensor.matmul(
                    po[:],
                    ot_sb[cc][:, st * 128 : (st + 1) * 128],
                    wo_sb[:, cc, dsl * 512 : (dsl + 1) * 512],
                    start=(cc == 0),
                    stop=(cc == 3),
                )
            if 3 in ccs:
                ob = ob_p.tile([128, 512], F16, tag="ob", name="ob")
                nc.vector.tensor_copy(ob[:], po[:])
                nc.sync.dma_start(
                    out=out_d[
                        st * 128 : (st + 1) * 128, dsl * 512 : (dsl + 1) * 512
                    ],
                    in_=ob[:],
                )
            return po

        def gen_oproj(j):
            for st in range(4 * j, 4 * j + 4):
                ob = ob_p.tile([128, D], F16, tag="ob", name="ob")
                for dsl in range(2):
                    po = ps_proj.tile([128, 512], F32, tag="ps", name="po")
                    for cc in range(4):
                        nc.tensor.matmul(
                            po[:],
                            ot_sb[cc][:, st * 128 : (st + 1) * 128],
                            wo_sb[:, cc, dsl * 512 : (dsl + 1) * 512],
                            start=(cc == 0),
                            stop=(cc == 3),
                        )
                        if cc < 3:
                            yield
                    nc.vector.tensor_copy(
                        ob[:, dsl * 512 : (dsl + 1) * 512], po[:]
                    )
                    yield
                nc.sync.dma_start(
                    out=out_d[st * 128 : (st + 1) * 128, :], in_=ob[:]
                )

        fillers = deque()  # (deadline, release, gen), deadline-ordered FIFO
        cur_slice = [(0, 0)]

        def pump(n):
            # advance the front filler n units; respect release gates so a
            # filler whose input DMA hasn't landed can't stall the in-order
            # PE mid-attention
            done = 0
            while done < n and fillers:
                _, rel, g = fillers[0]
                if rel > cur_slice[0]:
                    return
                try:
                    next(g)
                    done += 1
                except StopIteration:
                    fillers.popleft()

        def flush(cj):
            while fillers and fillers[0][0] <= cj:
                _, _, g = fillers[0]
                for _ in g:
                    pass
                fillers.popleft()

        # startup: weave V st0-2 + Q sl0 at d-granularity across four PSUM
        # slots (then V st3 + K sl0 over the wk pieces) so PE stays fed
        # while the first ~4 MB of DMA stream in
        def v_evac(ps, st):
            vt = v_p.tile([128, GH, 65], F16, tag="v", name=f"v{st}")
            nc.vector.memset(vt[:, :, 64:65], 1.0)
            nc.vector.tensor_add(
                vt[:, :, 0:64],
                ps[:].rearrange("p (h e) -> p h e", h=GH),
                bvb_sb[:].rearrange("p (h e) -> p h e", h=GH),
            )
            v_sb[st] = vt

        vps = [ps_proj.tile([128, 512], F32, tag="ps", name="ps") for _ in range(2)]
        vps += [ps_qk.tile([128, 512], F32, tag="qk", name="vqk") for _ in range(2)]
        qps = ps_pv.tile([128, 512], F32, tag="pv", name="qpv")
        for d in range(nd):
            for st in range(4):
                nc.tensor.matmul(
                    vps[st][:],
                    xt_sb[0][:, d, st * 128 : st * 128 + 128],
                    wv_sb[:, d, :],
                    start=(d == 0),
                    stop=(d == nd - 1),
                )
            nc.tensor.matmul(
                qps[:], wq_sb[:, d, 0:128], xt_sb[0][:, d, :],
                start=(d == 0), stop=(d == nd - 1),
            )
        for st in range(4):
            v_evac(vps[st], st)
        nc.vector.tensor_scalar_add(qt_sb[0][:, 0:512], qps[:], bqk_sb[:, 0:1])
        kps = ps_proj.tile([128, 512], F32, tag="ps", name="ps")
        for d in range(nd):
            nc.tensor.matmul(
                kps[:], wk_sb[:, d, 0:128], xt_sb[0][:, d, :],
                start=(d == 0), stop=(d == nd - 1),
            )
        nc.vector.tensor_scalar_add(kt_sb[0][:, 0:512], kps[:], bqk_sb[:, 4:5])
        nc.vector.tensor_scalar_add(kt_sb[0][:, 0:512], kps[:], bqk_sb[:, 4:5])
        for c in range(1, 4):
            fillers.append(((0, c), (0, 0), gen_qkproj(c, 0)))
        for j in range(1, ns):
            rel = (j - 1, 2)  # not before the xt block's DMA is due
            for st in range(4 * j, 4 * j + 4):
                fillers.append(((j, 0), rel, gen_vproj(st)))
            for c in range(4):
                fillers.append(((j, c), rel, gen_qkproj(c, j)))

        def evac_views(pv, stage, rcp, ci=None):
            """APs for the normalization mul over all 4 chunks (ci=None) or a
            single 128-chunk ci."""
            if ci is None:
                pv_v = bass.AP(
                    tensor=pv.tensor, offset=pv.offset,
                    ap=[pv.ap[0], [512, 2], [65, 4], [1, 64]],
                )
                st_v = bass.AP(
                    tensor=stage.tensor, offset=stage.offset,
                    ap=[stage.ap[0], [64, 2], [128, 4], [1, 64]],
                )
                rc_v = bass.AP(
                    tensor=rcp.tensor, offset=rcp.offset,
                    ap=[rcp.ap[0], [4, 2], [1, 4], [0, 64]],
                )
            else:
                pv_v = bass.AP(
                    tensor=pv.tensor, offset=pv.offset + ci * 65,
                    ap=[pv.ap[0], [512, 2], [1, 64]],
                )
                st_v = bass.AP(
                    tensor=stage.tensor, offset=stage.offset + ci * 128,
                    ap=[stage.ap[0], [64, 2], [1, 64]],
                )
                rc_v = bass.AP(
                    tensor=rcp.tensor, offset=rcp.offset + ci,
                    ap=[rcp.ap[0], [4, 2], [0, 64]],
                )
            return st_v, pv_v, rc_v

        prev_evac = [None]

        def emit_prev_evac():
            if prev_evac[0] is not None:
                prev_evac[0]()
                prev_evac[0] = None

        def attention(c, j):
            final = c == 3 and j == ns - 1
            cur_slice[0] = (j, c)
            # pv psum [128, 2, 512]: head hh in its own bank; chunk ci
            # occupies words ci*65..ci*65+65 (never straddles a bank).
            pv = ps_pv.tile([128, 2, 512], F32, tag="pv", name="pv")
            last = 4 * j + 3

            def emit_qk(t):
                diag = t >= 4 * j
                w0 = 128 * (t - 4 * j) if diag else 0
                qk = ps_qk.tile([128, 2, 512], F32, tag="qk", name="qk")
                for hh in range(2):
                    nc.tensor.matmul(
                        qk[:, hh, w0:512],
                        kt_sb[c][hh * 64 : hh * 64 + 64, t * 128 : (t + 1) * 128],
                        qt_sb[c][
                            hh * 64 : hh * 64 + 64, j * 512 + w0 : (j + 1) * 512
                        ],
                        start=True,
                        stop=True,
                    )
                et = et_p.tile([128, 2, 512], F16, tag="et", name="et")
                nc.scalar.activation(
                    et[:, :, w0:512], qk[:, :, w0:512], AF.Exp, scale=0.125
                )
                if diag:
                    for hh in range(2):
                        nc.vector.tensor_mul(
                            et[:, hh, w0 : w0 + 128],
                            et[:, hh, w0 : w0 + 128],
                            mask_sb[:],
                        )
                return et, w0

            def emit_pv(t, et, w0):
                # transposed PV: stationary = dense et chunk, moving = V.
                # PSUM zeroing is armed per 2KB bank: only the bank's first
                # matmul (ci=0, t=0) sets start; the other chunks' first
                # writes consume the bank-wide pending-zero.
                for ci in range(w0 // 128, 4):
                    for hh in range(2):
                        nc.tensor.matmul(
                            pv[:, hh, ci * 65 : ci * 65 + 65],
                            et[:, hh, ci * 128 : (ci + 1) * 128],
                            v_sb[t][:, 2 * c + hh, 0:65],
                            start=(t == 0 and ci == 0),
                            stop=(t == last),
                            skip_group_check=True,
                        )

            pend = emit_qk(0)
            emit_prev_evac()
            rate = pump_n
            for t in range(1, last + 1):
                cur = emit_qk(t)
                emit_pv(t - 1, *pend)
                pend = cur
                pump(rate)
            emit_pv(last, *pend)

            # --- normalization: deferred into the next slice's stream ---
            rowsums = bass.AP(
                tensor=pv.tensor,
                offset=pv.offset + 64,
                ap=[pv.ap[0], [512, 2], [65, 4]],
            )

            if not final:
                def evac(c=c, j=j, pv=pv, rowsums=rowsums):
                    rcp = rc_p.tile([128, 2, 4], F32, tag="rcp", name="rcp")
                    stage = st_p.tile([128, 512], F16, tag="stage", name="stage")
                    pst = ps_proj.tile([128, 4, 128], F16, tag="ps", name="pst")
                    nc.vector.reciprocal(rcp[:], rowsums)
                    st_v, pv_v, rc_v = evac_views(pv, stage, rcp)
                    nc.vector.tensor_mul(st_v, pv_v, rc_v)
                    for ci in range(4):
                        nc.tensor.transpose(
                            pst[:, ci, :],
                            stage[:, ci * 128 : (ci + 1) * 128],
                            idn_sb[:],
                        )
                    nc.vector.tensor_copy(
                        ot_sb[c][:, j * 512 : (j + 1) * 512],
                        pst[:].rearrange("p a b -> p (a b)"),
                    )
                    if c == 3:
                        fillers.append(((9, 9), (0, 0), gen_oproj(j)))
                prev_evac[0] = evac
            else:
                # final slice: reciprocal first (DVE overlaps the flush),
                # drain fillers (an open filler PSUM group would deadlock
                # pst below), pre-accumulate cc=0..2 for the first s-tile
                # (one open group per pool, keeping a slot free for pst),
                # evacuate pair 3 per 128-chunk, close as chunks land
                rcp = rc_p.tile([128, 2, 4], F32, tag="rcp", name="rcp")
                nc.vector.reciprocal(rcp[:], rowsums)
                flush((9, 9))
                stage = st_p.tile([128, 512], F16, tag="stage", name="stage")
                pst = ps_proj.tile([128, 4, 128], F16, tag="ps", name="pst")
                pos = {}
                for dsl in range(2):
                    pp, ptag = [(ps_proj, "ps"), (ps_qk, "qk")][dsl]
                    pos[dsl] = out_proj(4 * j, dsl, pp, ptag, range(3))
                obs = {}
                for ci in range(4):
                    st_v, pv_v, rc_v = evac_views(pv, stage, rcp, ci)
                    nc.vector.tensor_mul(st_v, pv_v, rc_v)
                    nc.tensor.transpose(
                        pst[:, ci, :],
                        stage[:, ci * 128 : (ci + 1) * 128],
                        idn_sb[:],
                    )
                    nc.vector.tensor_copy(
                        ot_sb[c][:, (4 * j + ci) * 128 : (4 * j + ci + 1) * 128],
                        pst[:, ci, :],
                    )
                    st = 4 * j + ci
                    ob = ob_p.tile([128, D], F16, tag="ob", name="ob")
                    for dsl in range(2):
                        if ci == 0:
                            po = pos[dsl]
                            nc.tensor.matmul(
                                po[:],
                                ot_sb[3][:, st * 128 : (st + 1) * 128],
                                wo_sb[:, 3, dsl * 512 : (dsl + 1) * 512],
                                start=False,
                                stop=True,
                            )
                        else:
                            pp, ptag = [(ps_proj, "ps"), (ps_qk, "qk")][dsl]
                            po = pp.tile([128, 512], F32, tag=ptag, name="po")
                            for cc in range(4):
                                nc.tensor.matmul(
                                    po[:],
                                    ot_sb[cc][:, st * 128 : (st + 1) * 128],
                                    wo_sb[:, cc, dsl * 512 : (dsl + 1) * 512],
                                    start=(cc == 0),
                                    stop=(cc == 3),
                                )
                        nc.vector.tensor_copy(
                            ob[:, dsl * 512 : (dsl + 1) * 512], po[:]
                        )
                    nc.sync.dma_start(
                        out=out_d[st * 128 : (st + 1) * 128, :], in_=ob[:]
                    )

        for j in range(ns):
            for c in range(4):
                flush((j, c))
                attention(c, j)
        flush((9, 9))

    if legalize:
        _legalize_waits(nc)
    return nc


_NC_CACHE = {}


def _get_nc(s=S):
    if s not in _NC_CACHE:
        _NC_CACHE[s] = build_nc(s)
    return _NC_CACHE[s]


def make_inputs(X, Wq, bq, Wk, bk, Wv, bv, Wo, bo, s=S):
    """Per-core input maps. Core c: batch c//2, head group c%2."""
    iv, jv = np.arange(128)[:, None], np.arange(128)[None, :]
    mask = (jv >= iv).astype(np.float16)
    idn = np.eye(128, dtype=np.float16)
    in_maps = []
    for c in range(8):
        b, g = divmod(c, 2)
        lo, hi = g * GW, (g + 1) * GW
        bqk = np.concatenate(
            [
                np.ascontiguousarray(bq[lo:hi].reshape(4, 128).T),
                np.ascontiguousarray(bk[lo:hi].reshape(4, 128).T),
            ],
            axis=1,
        ).astype(np.float32)
        in_maps.append(
            {
                "xt": np.ascontiguousarray(X[b, :s].T).astype(np.float16),
                "wq": np.ascontiguousarray(Wq[lo:hi].T).astype(np.float16),
                "wk": np.ascontiguousarray(Wk[lo:hi].T).astype(np.float16),
                "wv": np.ascontiguousarray(Wv[lo:hi].T).astype(np.float16),
                "wo": np.ascontiguousarray(Wo[:, lo:hi].T).astype(np.float16),
                "bqk": bqk,
                "bvb": np.tile(bv[lo:hi].astype(np.float16), (128, 1)),
                "mask": mask,
                "idn": idn,
            }
        )
    return in_maps


def kernel(X, Wq, bq, Wk, bk, Wv, bv, Wo, bo, **run_kwargs):
    args = [np.asarray(a, np.float32) for a in (X, Wq, bq, Wk, bk, Wv, bv, Wo, bo)]
    X, Wq, bq, Wk, bk, Wv, bv, Wo, bo = args
    nc = _get_nc(S)
    in_maps = make_inputs(X, Wq, bq, Wk, bk, Wv, bv, Wo, bo, S)
    res = run_bass_kernel_spmd(nc, in_maps, core_ids=list(range(8)), **run_kwargs)
    outs = [r["out"] for r in res.results]
    full = np.empty((B, S, D), np.float32)
    for b in range(B):
        full[b] = outs[2 * b].astype(np.float32) + outs[2 * b + 1] + bo
    kernel.last_results = res
    return full


# revision 37
# speedup vs baseline: 1.2421x; 1.0404x over previous
"""Multi-head causal attention (B=4, S=2048, D=1024, H=16) on 8 NeuronCores.

Sharding: core c handles batch b = c//2 and head-group g = c%2 (8 heads).
Each core computes QKV projections for its group, causal attention for its
8 heads, and a partial output projection (row-split Wo).  Host sums the two
fp16 partials per batch and adds bo.

On-chip design (per core), all matmuls fp16 with fp32 PSUM accumulation.
Engines execute in-order, so the emission IS the schedule: the attention
t-loop is ACT-bound (exp ~1.04us/t vs ~0.65us/t of PE work), so projection
work is decomposed into single-matmul "filler" units woven ~2 per t-step
into the attention stream, qk(t+1) is emitted before pv(t) so the wait on
exp(t) lands as late as possible, and each slice's normalization is
deferred into the next slice's stream.

  xt [D, S] = X[b].T in four [128, 8, 512] column blocks (block 0 and wv
  stream as interleaved per-d pieces so compute starts after ~1 MB of DMA).
  QT/KT pair-tiles [128, S]: partitions 0-63 = head 2c, 64-127 = head 2c+1.
  V per s-chunk [128, 8, 65]: 64 V cols (+bv) plus a ones column so every
  PV matmul emits rowsum(exp(scores)) in output column 64.
  scoresT tiles [sk=128, sq<=512] via two K=64 matmuls; windowed matmuls
  skip fully-masked regions; diagonal 128x128 blocks masked by a 0/1
  tensor_mul on the exp output.
  PV is TRANSPOSED vs the classic layout: stationary = dense et chunk
  [sk=128, sq=128], moving = V [sk=128, 65], accumulating psum pv[sq, 65]
  per (head, sq-chunk) over k-tiles: 65 moving columns per tile instead of
  512 (LdWeights is pipelined).  Rowsums land at pv col 64 on the same
  partition as their query, so normalization is one reciprocal [128, 2, 4]
  plus one broadcast tensor_mul per (pair, slice) into a normalized fp16
  stage [128, 512] = [sq, headA|headB]; PE transposes flip each 128-chunk
  into [hd-pair, sq] PSUM and one DVE copy lands ot_sb[c][:, slice] - the
  stationary layout the output projection needs.
  Output projection tiles run as fillers once their slice's four pairs are
  done; the final slice pre-accumulates cc=0..2 and fine-grains pair 3's
  evacuation per 128-chunk to shorten the tail.  Partials stored fp16.

Walrus wait-slot legality (1 sem wait per ACT/DVE/DMA instruction): touch
ops pre-observe constant DMAs and a legalization pass splits excess waits
onto same-engine NoOps.
"""

import sys

for _p in ("/opt/trn_rl_repo",):
    if _p not in sys.path:
        sys.path.insert(0, _p)

from collections import deque
from contextlib import ExitStack

import numpy as np
import ml_dtypes

import concourse.bass as bass
import concourse.mybir as mybir
import concourse.tile as tile
from concourse.bass_utils import run_bass_kernel_spmd

import bass_rust

F16 = mybir.dt.float16
F32 = mybir.dt.float32
F8 = mybir.dt.float8e4
DR = mybir.MatmulPerfMode.DoubleRow
TERMS = ((0, 0), (1, 1), (2, 0))  # (w-copy, x-copy): W8*X8 + W8h*R8 + S8*X8
AF = mybir.ActivationFunctionType

B, S, D, H = 4, 2048, 1024, 16
HD = D // H  # 64
GH = 8  # heads per group
GW = GH * HD  # 512 columns per group


_SPLITTABLE = {
    "InstMatmult", "InstLdweights", "InstActivation", "InstTensorCopy",
    "InstTensorTensor", "InstTensorScalarPtr", "InstTensorReduce",
    "InstMemset", "InstDMACopy", "InstReciprocal", "InstIota",
    "InstTensorTensorReduce", "InstBNStats", "InstBNStatsAggregate",
    "InstStreamShuffle", "InstNoOp", "InstPool", "InstMax", "InstDrain",
}


def _legalize_waits(nc, max_waits=1):
    """Walrus codegen accepts at most one sync-wait command per engine
    instruction; Tile's wait assigner can emit more.  Split extras onto
    same-engine NoOps inserted immediately before (semantics preserved:
    the engine blocks at the same program point)."""
    ctr = 0
    for fn in nc.m.functions:
        for blk in fn.blocks:
            out = []
            for ins in blk.instructions:
                si = ins.sync_info
                if (
                    si is not None
                    and len(si.on_wait) > max_waits
                    and type(ins).__name__ in _SPLITTABLE
                ):
                    waits = list(si.on_wait)
                    extra, keep = waits[:-max_waits], waits[-max_waits:]
                    for w in extra:
                        nop = mybir.InstNoOp(name=f"waitnop-{ctr}", ins=[], outs=[])
                        ctr += 1
                        nop.engine = ins.engine
                        nop.sync_info = bass_rust.SyncInfo(on_wait=[w], on_update=[])
                        out.append(nop)
                    ins.sync_info = bass_rust.SyncInfo(
                        on_wait=keep, on_update=list(si.on_update)
                    )
                out.append(ins)
            blk.instructions[:] = out
    return ctr


def build_nc(s=S, legalize=True, pump_n=2):
    ns = s // 512  # 512-wide sq slices per head
    nt = s // 128  # 128-wide k tiles
    nd = D // 128  # contraction chunks for projections
    nb = s // 512  # xt column blocks

    nc = bass.Bass("TRN2", target_bir_lowering=False, debug=False)
    xt_d = nc.dram_tensor("xt", [128, 4, 2, 2, s], F8, kind="ExternalInput").ap()
    wq_d = nc.dram_tensor("wq", [128, 4, 2, 3, GW], F8, kind="ExternalInput").ap()
    wk_d = nc.dram_tensor("wk", [128, 4, 2, 3, GW], F8, kind="ExternalInput").ap()
    wv_d = nc.dram_tensor("wv", [128, 4, 2, 3, GW], F8, kind="ExternalInput").ap()
    wo_d = nc.dram_tensor("wo", [GW, D], F16, kind="ExternalInput").ap()
    bqk_d = nc.dram_tensor("bqk", [128, 8], F32, kind="ExternalInput").ap()
    bvb_d = nc.dram_tensor("bvb", [128, GW], F16, kind="ExternalInput").ap()
    mask_d = nc.dram_tensor("mask", [128, 128], F16, kind="ExternalInput").ap()
    idn_d = nc.dram_tensor("idn", [128, 128], F16, kind="ExternalInput").ap()
    out_d = nc.dram_tensor("out", [s, D], F16, kind="ExternalOutput").ap()

    with tile.TileContext(nc) as tc, ExitStack() as ctx:
        pool = lambda name, bufs, **kw: ctx.enter_context(
            tc.tile_pool(name=name, bufs=bufs, **kw)
        )
        const_p = pool("const", 1)
        xt_p = pool("xtp", nb)
        w_p = pool("wp", 1)
        qt_p = pool("qtp", 4)
        kt_p = pool("ktp", 4)
        v_p = pool("vp", nt)
        et_p = pool("etp", 6)
        ot_p = pool("otp", 4)
        st_p = pool("stp", 2)
        rc_p = pool("rcp", 2)
        ob_p = pool("obp", 4)
        ps_proj = pool("psproj", 2, space="PSUM")  # [128,512]f32    -> 2 banks
        ps_qk = pool("psqk", 2, space="PSUM")      # [128,2,512]f32  -> 4 banks
        ps_pv = pool("pspv", 1, space="PSUM")      # [128,2,512]f32  -> 2 banks

        # --- input DMAs, ordered by first use.  wv and xt block 0 stream as
        # interleaved per-d pieces so V-proj st0's d-th matmul can fire as
        # soon as its two pieces land. ---
        # fp8 DoubleRow layouts: [partition, double-chunk, k-tile, copy, n]
        wv_sb = w_p.tile([128, 4, 2, 3, GW], F8)
        xt_sb = [
            xt_p.tile([128, 4, 2, 2, 512], F8, tag="xt", name=f"xtb{b}")
            for b in range(nb)
        ]
        bvb_sb = const_p.tile([128, GW], F16)
        wq_sb = w_p.tile([128, 4, 2, 3, GW], F8)
        wk_sb = w_p.tile([128, 4, 2, 3, GW], F8)
        wo_sb = w_p.tile([128, 4, D], F16)
        bqk_sb = const_p.tile([128, 8], F32)
        mask_sb = const_p.tile([128, 128], F16)
        idn_sb = const_p.tile([128, 128], F16)

        # graduated piece sizes: each dma_start costs ~0.5us of serialized
        # SP sequencer time, so only the leading pieces are fine-grained
        def dma_w(sb, dram, a, b, t=None):
            if t is None:
                nc.sync.dma_start(out=sb[:, a:b], in_=dram[:, a:b])
            else:
                nc.sync.dma_start(
                    out=sb[:, a:b, :, t, :], in_=dram[:, a:b, :, t, :]
                )

        # W8/X8 copies first (main terms), then W8h/R8, then S8 - matching
        # the term-major startup weave
        xt0_d = xt_d[:, :, :, :, 0:512]
        for a, b in ((0, 1), (1, 2), (2, 4)):
            dma_w(wv_sb, wv_d, a, b, t=0)
            dma_w(xt_sb[0], xt0_d, a, b, t=0)
        dma_w(wq_sb, wq_d, 0, 4, t=0)
        dma_w(wk_sb, wk_d, 0, 4, t=0)
        dma_w(wv_sb, wv_d, 0, 4, t=1)
        dma_w(xt_sb[0], xt0_d, 0, 4, t=1)
        dma_w(wq_sb, wq_d, 0, 4, t=1)
        dma_w(wk_sb, wk_d, 0, 4, t=1)
        dma_w(wv_sb, wv_d, 0, 4, t=2)
        dma_w(wq_sb, wq_d, 0, 4, t=2)
        dma_w(wk_sb, wk_d, 0, 4, t=2)
        small = [(bqk_sb, bqk_d), (bvb_sb, bvb_d), (mask_sb, mask_d),
                 (idn_sb, idn_d)]
        for t, src2 in small:
            nc.sync.dma_start(out=t[:], in_=src2[:])
        for bI in range(1, nb):
            nc.sync.dma_start(
                out=xt_sb[bI][:],
                in_=xt_d[:, :, :, :, bI * 512 : (bI + 1) * 512],
            )
        nc.sync.dma_start(out=wo_sb[:], in_=wo_d.rearrange("(c p) n -> p c n", p=128))

        # touch ops: early Exp-table load + const observations
        scr_a = const_p.tile([128, 1], F32)
        nc.scalar.activation(scr_a[:], bqk_sb[:, 0:1], AF.Exp)
        scr_v = const_p.tile([128, 1], F16)
        nc.vector.tensor_copy(scr_v[:], bvb_sb[:, 0:1])
        scr_m = const_p.tile([128, 1], F16)
        nc.vector.tensor_copy(scr_m[:], mask_sb[:, 0:1])
        scr_i = const_p.tile([128, 1], F16)
        nc.vector.tensor_copy(scr_i[:], idn_sb[:, 0:1])

        qt_sb = [qt_p.tile([128, s], F16, tag="qt", name=f"qt{c}") for c in range(4)]
        kt_sb = [kt_p.tile([128, s], F16, tag="kt", name=f"kt{c}") for c in range(4)]
        ot_sb = [ot_p.tile([128, s], F16, tag="ot", name=f"ot{c}") for c in range(4)]
        v_sb = [None] * nt

        # ---------- filler generators: one PE matmul (~0.2us) per yield ----
        def gen_vproj(st):
            ps = ps_proj.tile([128, 512], F32, tag="ps", name="ps")
            blk, col = st // 4, (st % 4) * 128
            for i, (d2, (wt, xs)) in enumerate(
                (d2, t) for d2 in range(4) for t in TERMS
            ):
                nc.tensor.matmul(
                    ps[:],
                    xt_sb[blk][:, d2, :, xs, col : col + 128],
                    wv_sb[:, d2, :, wt, :],
                    start=(i == 0), stop=(i == 11), perf_mode=DR,
                )
                if i % 2 == 1 and i < 11:
                    yield
            vt = v_p.tile([128, GH, 65], F16, tag="v", name=f"v{st}")
            nc.vector.memset(vt[:, :, 64:65], 32.0)
            nc.vector.tensor_add(
                vt[:, :, 0:64],
                ps[:].rearrange("p (h e) -> p h e", h=GH),
                bvb_sb[:].rearrange("p (h e) -> p h e", h=GH),
            )
            v_sb[st] = vt

        def gen_qkproj(c, sl):
            for dst, wsb, bcol in ((qt_sb[c], wq_sb, c), (kt_sb[c], wk_sb, 4 + c)):
                ps = ps_proj.tile([128, 512], F32, tag="ps", name="ps")
                for i, (d2, (wt, xs)) in enumerate(
                    (d2, t) for d2 in range(4) for t in TERMS
                ):
                    nc.tensor.matmul(
                        ps[:],
                        wsb[:, d2, :, wt, c * 128 : (c + 1) * 128],
                        xt_sb[sl][:, d2, :, xs, :],
                        start=(i == 0), stop=(i == 11), perf_mode=DR,
                    )
                    if i % 2 == 1 and i < 11:
                        yield
                nc.vector.tensor_scalar_add(
                    dst[:, sl * 512 : (sl + 1) * 512],
                    ps[:],
                    bqk_sb[:, bcol : bcol + 1],
                )
                yield

        def out_proj(st, dsl, pp, ptag, ccs, po=None):
            """Accumulate output projection for s-tile st, D-half dsl over
            pair-chunks ccs; evacuate + DMA when 3 in ccs."""
            if po is None:
                po = pp.tile([128, 512], F32, tag=ptag, name="po")
            for cc in ccs:
                nc.tensor.matmul(
                    po[:],
                    ot_sb[cc][:, st * 128 : (st + 1) * 128],
                    wo_sb[:, cc, dsl * 512 : (dsl + 1) * 512],
                    start=(cc == 0),
                    stop=(cc == 3),
                )
            if 3 in ccs:
                ob = ob_p.tile([128, 512], F16, tag="ob", name="ob")
                nc.vector.tensor_copy(ob[:], po[:])
                nc.sync.dma_start(
                    out=out_d[
                        st * 128 : (st + 1) * 128, dsl * 512 : (dsl + 1) * 512
                    ],
                    in_=ob[:],
                )
            return po

        def gen_oproj(j):
            for st in range(4 * j, 4 * j + 4):
                ob = ob_p.tile([128, D], F16, tag="ob", name="ob")
                for dsl in range(2):
                    po = ps_proj.tile([128, 512], F32, tag="ps", name="po")
                    for cc in range(4):
                        nc.tensor.matmul(
                            po[:],
                            ot_sb[cc][:, st * 128 : (st + 1) * 128],
                            wo_sb[:, cc, dsl * 512 : (dsl + 1) * 512],
                            start=(cc == 0),
                            stop=(cc == 3),
                        )
                        if cc < 3:
                            yield
                    nc.vector.tensor_copy(
                        ob[:, dsl * 512 : (dsl + 1) * 512], po[:]
                    )
                    yield
                nc.sync.dma_start(
                    out=out_d[st * 128 : (st + 1) * 128, :], in_=ob[:]
                )

        fillers = deque()  # (deadline, release, gen), deadline-ordered FIFO
        cur_slice = [(0, 0)]

        def pump(n):
            # advance the front filler n units; respect release gates so a
            # filler whose input DMA hasn't landed can't stall the in-order
            # PE mid-attention
            done = 0
            while done < n and fillers:
                _, rel, g = fillers[0]
                if rel > cur_slice[0]:
                    return
                try:
                    next(g)
                    done += 1
                except StopIteration:
                    fillers.popleft()

        def flush(cj):
            while fillers and fillers[0][0] <= cj:
                _, _, g = fillers[0]
                for _ in g:
                    pass
                fillers.popleft()

        # startup: weave V st0-2 + Q sl0 at d-granularity across four PSUM
        # slots (then V st3 + K sl0 over the wk pieces) so PE stays fed
        # while the first ~4 MB of DMA stream in
        def v_evac(ps, st):
            vt = v_p.tile([128, GH, 65], F16, tag="v", name=f"v{st}")
            nc.vector.memset(vt[:, :, 64:65], 32.0)
            nc.vector.tensor_add(
                vt[:, :, 0:64],
                ps[:].rearrange("p (h e) -> p h e", h=GH),
                bvb_sb[:].rearrange("p (h e) -> p h e", h=GH),
            )
            v_sb[st] = vt

        vps = [ps_proj.tile([128, 512], F32, tag="ps", name="ps") for _ in range(2)]
        vps += [ps_qk.tile([128, 512], F32, tag="qk", name="vqk")]
        qps = ps_pv.tile([128, 512], F32, tag="pv", name="qpv")
        kps = ps_qk.tile([128, 512], F32, tag="qk", name="vqk")
        for ti, (wt, xs) in enumerate(TERMS):
            first, last_t = ti == 0, ti == 2
            for st in range(3):
                for d2 in range(4):
                    nc.tensor.matmul(
                        vps[st][:],
                        xt_sb[0][:, d2, :, xs, st * 128 : st * 128 + 128],
                        wv_sb[:, d2, :, wt, :],
                        start=(first and d2 == 0), stop=(last_t and d2 == 3),
                        perf_mode=DR,
                    )
            for ps_t, wsb in ((qps, wq_sb), (kps, wk_sb)):
                for d2 in range(4):
                    nc.tensor.matmul(
                        ps_t[:],
                        wsb[:, d2, :, wt, 0:128],
                        xt_sb[0][:, d2, :, xs, :],
                        start=(first and d2 == 0), stop=(last_t and d2 == 3),
                        perf_mode=DR,
                    )
        for st in range(3):
            v_evac(vps[st], st)
        vp3 = ps_proj.tile([128, 512], F32, tag="ps", name="ps")
        for i, (d2, (wt, xs)) in enumerate(
            (d2, t) for t in TERMS for d2 in range(4)
        ):
            nc.tensor.matmul(
                vp3[:],
                xt_sb[0][:, d2, :, xs, 384:512],
                wv_sb[:, d2, :, wt, :],
                start=(i == 0), stop=(i == 11), perf_mode=DR,
            )
        v_evac(vp3, 3)
        nc.vector.tensor_scalar_add(qt_sb[0][:, 0:512], qps[:], bqk_sb[:, 0:1])
        nc.vector.tensor_scalar_add(kt_sb[0][:, 0:512], kps[:], bqk_sb[:, 4:5])
        nc.vector.tensor_scalar_add(kt_sb[0][:, 0:512], kps[:], bqk_sb[:, 4:5])
        nc.vector.tensor_scalar_add(kt_sb[0][:, 0:512], kps[:], bqk_sb[:, 4:5])
        nc.vector.tensor_scalar_add(kt_sb[0][:, 0:512], kps[:], bqk_sb[:, 4:5])
        for c in range(1, 4):
            fillers.append(((0, c), (0, 0), gen_qkproj(c, 0)))
        for j in range(1, ns):
            rel = (0, j)  # not before the xt block's DMA is due
            for st in range(4 * j, 4 * j + 4):
                fillers.append(((j, 0), rel, gen_vproj(st)))
            for c in range(4):
                fillers.append(((j, c), rel, gen_qkproj(c, j)))

        def evac_views(pv, stage, rcp, ci=None):
            """APs for the normalization mul over all 4 chunks (ci=None) or a
            single 128-chunk ci."""
            if ci is None:
                pv_v = bass.AP(
                    tensor=pv.tensor, offset=pv.offset,
                    ap=[pv.ap[0], [512, 2], [65, 4], [1, 64]],
                )
                st_v = bass.AP(
                    tensor=stage.tensor, offset=stage.offset,
                    ap=[stage.ap[0], [64, 2], [128, 4], [1, 64]],
                )
                rc_v = bass.AP(
                    tensor=rcp.tensor, offset=rcp.offset,
                    ap=[rcp.ap[0], [4, 2], [1, 4], [0, 64]],
                )
            else:
                pv_v = bass.AP(
                    tensor=pv.tensor, offset=pv.offset + ci * 65,
                    ap=[pv.ap[0], [512, 2], [1, 64]],
                )
                st_v = bass.AP(
                    tensor=stage.tensor, offset=stage.offset + ci * 128,
                    ap=[stage.ap[0], [64, 2], [1, 64]],
                )
                rc_v = bass.AP(
                    tensor=rcp.tensor, offset=rcp.offset + ci,
                    ap=[rcp.ap[0], [4, 2], [0, 64]],
                )
            return st_v, pv_v, rc_v

        prev_evac = [None]

        def emit_prev_evac():
            if prev_evac[0] is not None:
                prev_evac[0]()
                prev_evac[0] = None

        def attention(c, j):
            final = c == 3 and j == ns - 1
            cur_slice[0] = (j, c)
            # pv psum [128, 2, 512]: head hh in its own bank; chunk ci
            # occupies words ci*65..ci*65+65 (never straddles a bank).
            pv = ps_pv.tile([128, 2, 512], F32, tag="pv", name="pv")
            last = 4 * j + 3

            def emit_qk(t):
                diag = t >= 4 * j
                w0 = 128 * (t - 4 * j) if diag else 0
                qk = ps_qk.tile([128, 2, 512], F32, tag="qk", name="qk")
                for hh in range(2):
                    nc.tensor.matmul(
                        qk[:, hh, w0:512],
                        kt_sb[c][hh * 64 : hh * 64 + 64, t * 128 : (t + 1) * 128],
                        qt_sb[c][
                            hh * 64 : hh * 64 + 64, j * 512 + w0 : (j + 1) * 512
                        ],
                        start=True,
                        stop=True,
                    )
                et = et_p.tile([128, 2, 512], F16, tag="et", name="et")
                nc.scalar.activation(
                    et[:, :, w0:512], qk[:, :, w0:512], AF.Exp, scale=0.125 / 1024.0
                )
                if diag:
                    for hh in range(2):
                        nc.vector.tensor_mul(
                            et[:, hh, w0 : w0 + 128],
                            et[:, hh, w0 : w0 + 128],
                            mask_sb[:],
                        )
                return et, w0

            def emit_pv(t, et, w0):
                # transposed PV: stationary = dense et chunk, moving = V.
                # PSUM zeroing is armed per 2KB bank: only the bank's first
                # matmul (ci=0, t=0) sets start; the other chunks' first
                # writes consume the bank-wide pending-zero.
                for ci in range(w0 // 128, 4):
                    for hh in range(2):
                        nc.tensor.matmul(
                            pv[:, hh, ci * 65 : ci * 65 + 65],
                            et[:, hh, ci * 128 : (ci + 1) * 128],
                            v_sb[t][:, 2 * c + hh, 0:65],
                            start=(t == 0 and ci == 0),
                            stop=(t == last),
                            skip_group_check=True,
                        )

            pend = emit_qk(0)
            emit_prev_evac()
            rate = pump_n
            for t in range(1, last + 1):
                cur = emit_qk(t)
                emit_pv(t - 1, *pend)
                pend = cur
                pump(rate)
            emit_pv(last, *pend)

            # --- normalization: deferred into the next slice's stream ---
            rowsums = bass.AP(
                tensor=pv.tensor,
                offset=pv.offset + 64,
                ap=[pv.ap[0], [512, 2], [65, 4]],
            )

            if not final:
                def evac(c=c, j=j, pv=pv, rowsums=rowsums):
                    rcp = rc_p.tile([128, 2, 4], F32, tag="rcp", name="rcp")
                    stage = st_p.tile([128, 512], F16, tag="stage", name="stage")
                    pst = ps_proj.tile([128, 4, 128], F16, tag="ps", name="pst")
                    nc.vector.reciprocal(rcp[:], rowsums)
                    st_v, pv_v, rc_v = evac_views(pv, stage, rcp)
                    nc.vector.tensor_mul(st_v, pv_v, rc_v)
                    for ci in range(4):
                        nc.tensor.transpose(
                            pst[:, ci, :],
                            stage[:, ci * 128 : (ci + 1) * 128],
                            idn_sb[:],
                        )
                    nc.vector.tensor_copy(
                        ot_sb[c][:, j * 512 : (j + 1) * 512],
                        pst[:].rearrange("p a b -> p (a b)"),
                    )
                    if c == 3:
                        fillers.append(((9, 9), (0, 0), gen_oproj(j)))
                prev_evac[0] = evac
            else:
                # final slice: reciprocal first (DVE overlaps the flush),
                # drain fillers (an open filler PSUM group would deadlock
                # pst below), pre-accumulate cc=0..2 for the first s-tile
                # (one open group per pool, keeping a slot free for pst),
                # evacuate pair 3 per 128-chunk, close as chunks land
                rcp = rc_p.tile([128, 2, 4], F32, tag="rcp", name="rcp")
                nc.vector.reciprocal(rcp[:], rowsums)
                flush((9, 9))
                stage = st_p.tile([128, 512], F16, tag="stage", name="stage")
                pst = ps_proj.tile([128, 4, 128], F16, tag="ps", name="pst")
                pos = {}
                for dsl in range(2):
                    pp, ptag = [(ps_proj, "ps"), (ps_qk, "qk")][dsl]
                    pos[dsl] = out_proj(4 * j, dsl, pp, ptag, range(3))
                obs = {}
                for ci in range(4):
                    st_v, pv_v, rc_v = evac_views(pv, stage, rcp, ci)
                    nc.vector.tensor_mul(st_v, pv_v, rc_v)
                    nc.tensor.transpose(
                        pst[:, ci, :],
                        stage[:, ci * 128 : (ci + 1) * 128],
                        idn_sb[:],
                    )
                    nc.vector.tensor_copy(
                        ot_sb[c][:, (4 * j + ci) * 128 : (4 * j + ci + 1) * 128],
                        pst[:, ci, :],
                    )
                    st = 4 * j + ci
                    ob = ob_p.tile([128, D], F16, tag="ob", name="ob")
                    for dsl in range(2):
                        if ci == 0:
                            po = pos[dsl]
                            nc.tensor.matmul(
                                po[:],
                                ot_sb[3][:, st * 128 : (st + 1) * 128],
                                wo_sb[:, 3, dsl * 512 : (dsl + 1) * 512],
                                start=False,
                                stop=True,
                            )
                        else:
                            pp, ptag = [(ps_proj, "ps"), (ps_qk, "qk")][dsl]
                            po = pp.tile([128, 512], F32, tag=ptag, name="po")
                            for cc in range(4):
                                nc.tensor.matmul(
                                    po[:],
                                    ot_sb[cc][:, st * 128 : (st + 1) * 128],
                                    wo_sb[:, cc, dsl * 512 : (dsl + 1) * 512],
                                    start=(cc == 0),
                                    stop=(cc == 3),
                                )
                        nc.vector.tensor_copy(
                            ob[:, dsl * 512 : (dsl + 1) * 512], po[:]
                        )
                    nc.sync.dma_start(
                        out=out_d[st * 128 : (st + 1) * 128, :], in_=ob[:]
                    )

        for j in range(ns):
            for c in range(4):
                flush((j, c))
                attention(c, j)
        flush((9, 9))

    if legalize:
        _legalize_waits(nc)
    return nc


_NC_CACHE = {}


def _get_nc(s=S):
    if s not in _NC_CACHE:
        _NC_CACHE[s] = build_nc(s)
    return _NC_CACHE[s]


def make_inputs(X, Wq, bq, Wk, bk, Wv, bv, Wo, bo, s=S):
    """Per-core input maps. Core c: batch c//2, head group c%2."""
    iv, jv = np.arange(128)[:, None], np.arange(128)[None, :]
    mask = (jv >= iv).astype(np.float16)
    idn = np.eye(128, dtype=np.float16)

    def q8(x):
        return np.asarray(x, dtype=ml_dtypes.float8_e4m3fn)

    def lay(a):  # [D, n] -> [128, 4, 2, n]: D = d2*256 + kt*128 + p
        return np.ascontiguousarray(
            a.reshape(4, 2, 128, -1).transpose(2, 0, 1, 3)
        )

    def pack_w(Wt):  # [D, 512] fp32 -> [128, 4, 2, 3, 512] fp8
        W8 = q8(32 * Wt)
        return np.stack(
            [lay(W8), lay(q8(2 * Wt)), lay(q8(32 * Wt - W8.astype(np.float32)))],
            axis=3,
        )

    in_maps = []
    for c in range(8):
        b, g = divmod(c, 2)
        lo, hi = g * GW, (g + 1) * GW
        bqk = 32 * np.concatenate(
            [
                np.ascontiguousarray(bq[lo:hi].reshape(4, 128).T),
                np.ascontiguousarray(bk[lo:hi].reshape(4, 128).T),
            ],
            axis=1,
        ).astype(np.float32)
        Xb = np.ascontiguousarray(X[b, :s].T)
        X8 = q8(Xb)
        R8 = q8(16 * (Xb - X8.astype(np.float32)))
        in_maps.append(
            {
                "xt": np.stack([lay(X8), lay(R8)], axis=3),
                "wq": pack_w(np.ascontiguousarray(Wq[lo:hi].T)),
                "wk": pack_w(np.ascontiguousarray(Wk[lo:hi].T)),
                "wv": pack_w(np.ascontiguousarray(Wv[lo:hi].T)),
                "wo": np.ascontiguousarray(Wo[:, lo:hi].T).astype(np.float16),
                "bqk": bqk,
                "bvb": np.tile(32 * bv[lo:hi], (128, 1)).astype(np.float16),
                "mask": mask,
                "idn": idn,
            }
        )
    return in_maps


def kernel(X, Wq, bq, Wk, bk, Wv, bv, Wo, bo, **run_kwargs):
    args = [np.asarray(a, np.float32) for a in (X, Wq, bq, Wk, bk, Wv, bv, Wo, bo)]
    X, Wq, bq, Wk, bk, Wv, bv, Wo, bo = args
    nc = _get_nc(S)
    in_maps = make_inputs(X, Wq, bq, Wk, bk, Wv, bv, Wo, bo, S)
    res = run_bass_kernel_spmd(nc, in_maps, core_ids=list(range(8)), **run_kwargs)
    outs = [r["out"] for r in res.results]
    full = np.empty((B, S, D), np.float32)
    for b in range(B):
        full[b] = outs[2 * b].astype(np.float32) + outs[2 * b + 1] + bo
    kernel.last_results = res
    return full
